# revision 11
# baseline (speedup 1.0000x reference)
"""Trainium2 Bass kernel for one dense transformer block.

B=8, T=2048, C=768, 8 heads. Data-parallel: one batch element per NeuronCore,
no collectives. Full inputs in, full output out.
"""
import os
import numpy as np
import ml_dtypes

import concourse.bass as bass
import concourse.mybir as mybir
import concourse.tile as tile
from concourse import bacc
from contextlib import ExitStack

F32 = mybir.dt.float32
BF16 = mybir.dt.bfloat16
AF = mybir.ActivationFunctionType
OP = mybir.AluOpType

P = 128
T, C = 2048, 768
H, D = 8, 96
FC = 3072
NT = T // P          # 16 token tiles
NF = C // P          # 6 feature chunks
NFC = FC // P        # 24 fc feature tiles
LN_EPS = 1e-5
SM_SCALE = 1.0 / float(np.sqrt(D))


def _ln_tile(nc, pools, xi):
    """LayerNorm stats + normalize for one [128, C] f32 tile -> bf16 tile.

    Returns the normalized (x - mean) * rstd tile (gamma/beta applied later,
    per-feature, during the post-transpose eviction).
    """
    st6 = pools["st6"].tile([P, 2, 6], F32, tag="st6")
    nc.vector.bn_stats(st6[:, 0, :], xi[:, 0:C // 2])
    nc.vector.bn_stats(st6[:, 1, :], xi[:, C // 2:C])
    aggr = pools["aggr"].tile([P, 2], F32, tag="aggr")
    nc.vector.bn_aggr(aggr[:], st6[:])
    std = pools["std"].tile([P, 1], F32, tag="std")
    nc.scalar.activation(std[:], aggr[:, 1:2], AF.Sqrt, bias=pools["eps"][:])
    rstd = pools["rstd"].tile([P, 1], F32, tag="rstd")
    nc.vector.reciprocal(rstd[:], std[:])
    xln = pools["xln"].tile([P, C], BF16, tag="xln")
    nc.vector.tensor_scalar(out=xln[:], in0=xi, scalar1=aggr[:, 0:1],
                            scalar2=rstd[:], op0=OP.subtract, op1=OP.mult)
    return xln


def _ln_transpose_group(nc, tc, pools, xlns, g, xt_dst, ident, lnw, lnb):
    """Transpose 4 normalized token tiles into xt_dst[:, f, g*512:(g+1)*512],
    applying per-feature gamma/beta during PSUM eviction."""
    for f in range(NF):
        pt = pools["tps"].tile([P, 512], BF16, tag="tps")
        for j in range(4):
            nc.tensor.transpose(pt[:, j * P:(j + 1) * P],
                                xlns[j][:, f * P:(f + 1) * P], ident)
        nc.vector.tensor_scalar(out=xt_dst[:, f, g * 512:(g + 1) * 512],
                                in0=pt[:], scalar1=lnw[:, f:f + 1],
                                scalar2=lnb[:, f:f + 1], op0=OP.mult, op1=OP.add)


def build_nc(repeat=1, debug=False):
    nc = bacc.Bacc(None, target_bir_lowering=False)
    with tile.TileContext(nc) as tc, ExitStack() as top:
        dram = top.enter_context(tc.tile_pool(name="dram", bufs=1, space="DRAM"))

        def din(name, shape, dt):
            return dram.tile(shape, dt, kind="ExternalInput", name=name, uniquify=False)

        x_d = din("x", [T, C], F32)
        wqkvT_d = din("wqkvT", [C, 3 * C], BF16)       # w_qkv.T
        wa_d = din("waT", [H * P, C], BF16)            # w_attn_proj.T head-padded
        wf_d = din("wfT", [C, FC], BF16)               # w_fc.T
        wm_d = din("wmT", [FC, C], BF16)               # w_mlp_proj.T
        bqk_d = din("bqk", [P, 16], F32)               # per head-block bias cols
        bvb_d = din("bvb", [P, C], F32)                # b_v broadcast
        ba_d = din("ba", [P, NF], F32)
        bf_d = din("bf", [P, NFC], F32)
        bm_d = din("bm", [P, NF], F32)
        l1w_d = din("l1w", [P, NF], F32)
        l1b_d = din("l1b", [P, NF], F32)
        l2w_d = din("l2w", [P, NF], F32)
        l2b_d = din("l2b", [P, NF], F32)
        tri_d = din("tri", [P, P], BF16)
        id_d = din("ident", [P, P], BF16)
        out_d = dram.tile([T, C], F32, kind="ExternalOutput", name="out", uniquify=False)
        xp_d = dram.tile([T, C], F32, name="xprime", uniquify=False)
        if debug:
            dbg_x1t = dram.tile([P, NF, T], BF16, kind="ExternalOutput", name="dbg_x1t", uniquify=False)
            dbg_qk = dram.tile([P, 16, T], BF16, kind="ExternalOutput", name="dbg_qk", uniquify=False)
            dbg_v = dram.tile([P, NT, H, 97], BF16, kind="ExternalOutput", name="dbg_v", uniquify=False)
            dbg_o = dram.tile([P, H, T], BF16, kind="ExternalOutput", name="dbg_o", uniquify=False)

        xv = x_d[:].rearrange("(n p) c -> n p c", p=P)
        xpv = xp_d[:].rearrange("(n p) c -> n p c", p=P)
        outv = out_d[:].rearrange("(n p) c -> n p c", p=P)

        # constants / params, loaded once
        cst = top.enter_context(tc.tile_pool(name="cst", bufs=1))
        ident = cst.tile([P, P], BF16)
        nc.sync.dma_start(out=ident[:], in_=id_d[:])
        tri = cst.tile([P, P], BF16)
        nc.sync.dma_start(out=tri[:], in_=tri_d[:])
        bqk = cst.tile([P, 16], F32)
        nc.sync.dma_start(out=bqk[:], in_=bqk_d[:])
        bvb = cst.tile([P, C], F32)
        nc.sync.dma_start(out=bvb[:], in_=bvb_d[:])
        ba = cst.tile([P, NF], F32)
        nc.sync.dma_start(out=ba[:], in_=ba_d[:])
        bf = cst.tile([P, NFC], F32)
        nc.sync.dma_start(out=bf[:], in_=bf_d[:])
        bm = cst.tile([P, NF], F32)
        nc.sync.dma_start(out=bm[:], in_=bm_d[:])
        l1w = cst.tile([P, NF], F32)
        nc.sync.dma_start(out=l1w[:], in_=l1w_d[:])
        l1b = cst.tile([P, NF], F32)
        nc.sync.dma_start(out=l1b[:], in_=l1b_d[:])
        l2w = cst.tile([P, NF], F32)
        nc.sync.dma_start(out=l2w[:], in_=l2w_d[:])
        l2b = cst.tile([P, NF], F32)
        nc.sync.dma_start(out=l2b[:], in_=l2b_d[:])
        eps_t = cst.tile([P, 1], F32)
        nc.vector.memset(eps_t[:], LN_EPS)

        def body(iv):
            with ExitStack() as es:
                ln_pools = {
                    "st6": es.enter_context(tc.tile_pool(name="st6", bufs=4)),
                    "aggr": es.enter_context(tc.tile_pool(name="aggr", bufs=4)),
                    "std": es.enter_context(tc.tile_pool(name="std", bufs=4)),
                    "rstd": es.enter_context(tc.tile_pool(name="rstd", bufs=4)),
                    "xln": es.enter_context(tc.tile_pool(name="xln", bufs=6)),
                    "eps": eps_t,
                }
                xt_pool = es.enter_context(tc.tile_pool(name="xt", bufs=1))

                # ---------------- Phase A: LN1 + transpose ----------------
                x1t = xt_pool.tile([P, NF, T], BF16, tag="xt")
                with tc.tile_pool(name="xi_a", bufs=6) as xi_pool, \
                     tc.tile_pool(name="tps_a", bufs=2, space="PSUM") as tps_a:
                    ln_pools["tps"] = tps_a
                    for g in range(4):
                        xlns = []
                        for j in range(4):
                            i = g * 4 + j
                            xi = xi_pool.tile([P, C], F32, tag="xi")
                            nc.sync.dma_start(out=xi[:], in_=xv[i])
                            xlns.append(_ln_tile(nc, ln_pools, xi[:]))
                        _ln_transpose_group(nc, tc, ln_pools, xlns, g, x1t, ident, l1w, l1b)

                # ---------------- Phase B: QKV ----------------
                es_bc = ExitStack()
                qk_pool = es_bc.enter_context(tc.tile_pool(name="qk", bufs=1))
                v_pool = es_bc.enter_context(tc.tile_pool(name="v", bufs=1))
                qk_sb = qk_pool.tile([P, 16, T], BF16, tag="qk")
                v_sb = v_pool.tile([P, NT, H, 97], BF16, tag="v")
                if debug:
                    nc.gpsimd.memset(qk_sb[96:128, :, :], 0.0)
                nc.vector.memset(v_sb[:, :, :, 0], 1.0)
                with ExitStack() as eb:
                    wq_pool = eb.enter_context(tc.tile_pool(name="wq", bufs=1))
                    qkps_pool = eb.enter_context(tc.tile_pool(name="qkps", bufs=4, space="PSUM"))
                    vps_pool = eb.enter_context(tc.tile_pool(name="vps", bufs=2, space="PSUM"))
                    wq = wq_pool.tile([P, NF, 3 * C], BF16, tag="wq")
                    nc.sync.dma_start(out=wq[:], in_=wqkvT_d[:].rearrange("(f p) c -> p f c", p=P))
                    # B1: Q, K feature-major, head-padded blocks (M=96)
                    for m in range(16):
                        col0 = m * D if m < 8 else C + (m - 8) * D
                        for n in range(4):
                            ps = qkps_pool.tile([P, 512], F32, tag="qkps")
                            for f in range(NF):
                                nc.tensor.matmul(ps[0:D, :], lhsT=wq[:, f, col0:col0 + D],
                                                 rhs=x1t[:, f, n * 512:(n + 1) * 512],
                                                 start=(f == 0), stop=(f == NF - 1))
                            nc.scalar.activation(qk_sb[0:D, m, n * 512:(n + 1) * 512],
                                                 ps[0:D, :], AF.Identity,
                                                 bias=bqk[0:D, m:m + 1])
                    # B2: V token-major [tk, h, d] + ones column
                    for i in range(NT):
                        ps = vps_pool.tile([P, C], F32, tag="vps")
                        for f in range(NF):
                            nc.tensor.matmul(ps[:, 0:512], lhsT=x1t[:, f, i * P:(i + 1) * P],
                                             rhs=wq[:, f, 2 * C:2 * C + 512],
                                             start=(f == 0), stop=(f == NF - 1))
                            nc.tensor.matmul(ps[:, 512:768], lhsT=x1t[:, f, i * P:(i + 1) * P],
                                             rhs=wq[:, f, 2 * C + 512:3 * C],
                                             start=(f == 0), stop=(f == NF - 1))
                        nc.vector.tensor_tensor(
                            out=v_sb[:, i, :, 1:97],
                            in0=ps[:].rearrange("p (h d) -> p h d", h=H),
                            in1=bvb[:].rearrange("p (h d) -> p h d", h=H),
                            op=OP.add)

                if debug:
                    nc.sync.dma_start(out=dbg_x1t[:], in_=x1t[:])
                    nc.sync.dma_start(out=dbg_qk[:], in_=qk_sb[:])
                    nc.sync.dma_start(out=dbg_v[:], in_=v_sb[:])

                # ---------------- Phase C: attention ----------------
                es_cd = ExitStack()
                o_pool = es_cd.enter_context(tc.tile_pool(name="o", bufs=1, side="right"))
                o_sb = o_pool.tile([P, H, T], BF16, tag="o")
                nc.vector.memset(o_sb[96:128, :, :], 0.0)
                with ExitStack() as ec:
                    sps_pool = ec.enter_context(tc.tile_pool(name="sps", bufs=2, space="PSUM"))
                    ops_pool = ec.enter_context(tc.tile_pool(name="ops", bufs=1, space="PSUM"))
                    pt_pool = ec.enter_context(tc.tile_pool(name="pt", bufs=3))
                    r_pool = ec.enter_context(tc.tile_pool(name="r", bufs=1))
                    rb_pool = ec.enter_context(tc.tile_pool(name="rb", bufs=1))

                    for h in range(H):
                        o_ps = ops_pool.tile([P, T], F32, tag="ops")
                        for tk in range(NT):
                            q0 = tk * P
                            pT = pt_pool.tile([P, T], BF16, tag="pt")
                            # scores S^T[tk-tile, tq] and exp -> pT
                            for l in range(q0 // 1024, 2):
                                sb_ = sps_pool.tile([P, 1024], F32, tag="sps")
                                for c2 in range(2):
                                    cs = l * 1024 + c2 * 512
                                    if cs + 512 <= q0:
                                        continue
                                    nc.tensor.matmul(sb_[:, c2 * 512:(c2 + 1) * 512],
                                                     lhsT=qk_sb[0:D, 8 + h, q0:q0 + P],
                                                     rhs=qk_sb[0:D, h, cs:cs + 512],
                                                     start=True, stop=True)
                                e0 = max(q0 - l * 1024, 0)
                                nc.scalar.activation(pT[:, l * 1024 + e0:(l + 1) * 1024],
                                                     sb_[:, e0:1024], AF.Exp,
                                                     scale=SM_SCALE)
                            # causal mask on diagonal block
                            nc.vector.tensor_tensor(out=pT[:, q0:q0 + P], in0=pT[:, q0:q0 + P],
                                                    in1=tri[:], op=OP.mult)
                            # PV: O'[d(+sum), tq] += V'[tk].T @ pT, bank-aligned pieces
                            pos = q0
                            while pos < T:
                                bank = pos // 512
                                end = min((bank + 1) * 512, T)
                                nc.tensor.matmul(o_ps[0:97, pos:end],
                                                 lhsT=v_sb[:, tk, h, :],
                                                 rhs=pT[:, pos:end],
                                                 start=(tk == 0),
                                                 stop=(tk == min(4 * bank + 3, NT - 1)))
                                pos = end
                        # normalize by denominator row and evict
                        r_row = r_pool.tile([P, T], F32, tag="r")
                        nc.vector.reciprocal(r_row[0:1, :], o_ps[0:1, :])
                        rb = rb_pool.tile([P, T], F32, tag="rb")
                        nc.gpsimd.partition_broadcast(rb[0:97, :], r_row[0:1, :], channels=97)
                        nc.vector.tensor_tensor(out=o_sb[0:97, h, :], in0=o_ps[0:97, :],
                                                in1=rb[0:97, :], op=OP.mult)

                if debug:
                    nc.sync.dma_start(out=dbg_o[:], in_=o_sb[:])
                es_bc.close()

                # ---------------- Phase D: attn proj + residual + LN2 ----------------
                x2t = xt_pool.tile([P, NF, T], BF16, tag="xt")
                with ExitStack() as ed:
                    wa_pool2 = ed.enter_context(tc.tile_pool(name="wa2", bufs=1))
                    wa2 = wa_pool2.tile([P, H, C], BF16, tag="wa2")
                    nc.sync.dma_start(out=wa2[:], in_=wa_d[:].rearrange("(h p) c -> p h c", p=P))
                    prps_pool = ed.enter_context(tc.tile_pool(name="prps", bufs=2, space="PSUM"))
                    tmps_pool = ed.enter_context(tc.tile_pool(name="tmps", bufs=2, space="PSUM"))
                    stg_pool = ed.enter_context(tc.tile_pool(name="stg", bufs=2))
                    xi_pool = ed.enter_context(tc.tile_pool(name="xi_d", bufs=3))
                    xn_pool = ed.enter_context(tc.tile_pool(name="xn", bufs=6))
                    ln2_tps = ed.enter_context(tc.tile_pool(name="tps2", bufs=2, space="PSUM"))
                    ln_pools2 = dict(ln_pools)
                    ln_pools2["tps"] = ln2_tps

                    for n in range(4):
                        stg = stg_pool.tile([P, NF, 512], BF16, tag="stg")
                        for fo in range(NF):
                            ps = prps_pool.tile([P, 512], F32, tag="prps")
                            for h in range(H):
                                nc.tensor.matmul(ps[:], lhsT=wa2[:, h, fo * P:(fo + 1) * P],
                                                 rhs=o_sb[:, h, n * 512:(n + 1) * 512],
                                                 start=(h == 0), stop=(h == H - 1))
                            nc.scalar.activation(stg[:, fo, :], ps[:], AF.Identity,
                                                 bias=ba[:, fo:fo + 1])
                        xlns = []
                        for j in range(4):
                            i = n * 4 + j
                            tm = tmps_pool.tile([P, C], BF16, tag="tmps")
                            for fo in range(NF):
                                nc.tensor.transpose(tm[:, fo * P:(fo + 1) * P],
                                                    stg[:, fo, j * P:(j + 1) * P], ident)
                            xi = xi_pool.tile([P, C], F32, tag="xi")
                            nc.sync.dma_start(out=xi[:], in_=xv[i])
                            xn = xn_pool.tile([P, C], F32, tag="xn")
                            nc.vector.tensor_tensor(out=xn[:], in0=xi[:], in1=tm[:], op=OP.add)
                            nc.sync.dma_start(out=xpv[i], in_=xn[:])
                            xlns.append(_ln_tile(nc, ln_pools2, xn[:]))
                        _ln_transpose_group(nc, tc, ln_pools2, xlns, n, x2t, ident, l2w, l2b)
                es_cd.close()

                # ---------------- Phase E: MLP (T halves) ----------------
                with ExitStack() as ee:
                    wf_pool = ee.enter_context(tc.tile_pool(name="wf", bufs=1))
                    wm_pool = ee.enter_context(tc.tile_pool(name="wm", bufs=1))
                    wf = wf_pool.tile([P, NF, FC], BF16, tag="wf")
                    nc.sync.dma_start(out=wf[:], in_=wf_d[:].rearrange("(f p) c -> p f c", p=P))
                    wm = wm_pool.tile([P, NFC, C], BF16, tag="wm")
                    nc.sync.dma_start(out=wm[:], in_=wm_d[:].rearrange("(f p) c -> p f c", p=P))
                    h_pool = ee.enter_context(tc.tile_pool(name="h", bufs=1))
                    fcps_pool = ee.enter_context(tc.tile_pool(name="fcps", bufs=2, space="PSUM"))
                    prps2_pool = ee.enter_context(tc.tile_pool(name="prps2", bufs=2, space="PSUM"))
                    tmps2_pool = ee.enter_context(tc.tile_pool(name="tmps2", bufs=1, space="PSUM"))
                    stg2_pool = ee.enter_context(tc.tile_pool(name="stg2", bufs=2))
                    xi2_pool = ee.enter_context(tc.tile_pool(name="xi_e", bufs=3))
                    xo_pool = ee.enter_context(tc.tile_pool(name="xo", bufs=3))

                    for t2 in range(2):
                        tbase = t2 * 1024
                        h_sb = h_pool.tile([P, NFC, 1024], BF16, tag="h")
                        for m in range(NFC):
                            ps = fcps_pool.tile([P, 1024], F32, tag="fcps")
                            for c2 in range(2):
                                for f in range(NF):
                                    nc.tensor.matmul(
                                        ps[:, c2 * 512:(c2 + 1) * 512],
                                        lhsT=wf[:, f, m * P:(m + 1) * P],
                                        rhs=x2t[:, f, tbase + c2 * 512:tbase + (c2 + 1) * 512],
                                        start=(f == 0), stop=(f == NF - 1))
                            nc.scalar.activation(h_sb[:, m, :], ps[:], AF.Gelu_apprx_tanh,
                                                 bias=bf[:, m:m + 1])
                        for n2 in range(2):
                            stg = stg2_pool.tile([P, NF, 512], BF16, tag="stg2")
                            for fo in range(NF):
                                ps = prps2_pool.tile([P, 512], F32, tag="prps2")
                                for k in range(NFC):
                                    nc.tensor.matmul(ps[:], lhsT=wm[:, k, fo * P:(fo + 1) * P],
                                                     rhs=h_sb[:, k, n2 * 512:(n2 + 1) * 512],
                                                     start=(k == 0), stop=(k == NFC - 1))
                                nc.scalar.activation(stg[:, fo, :], ps[:], AF.Identity,
                                                     bias=bm[:, fo:fo + 1])
                            for j in range(4):
                                i = t2 * 8 + n2 * 4 + j
                                tm = tmps2_pool.tile([P, C], BF16, tag="tmps2")
                                for fo in range(NF):
                                    nc.tensor.transpose(tm[:, fo * P:(fo + 1) * P],
                                                        stg[:, fo, j * P:(j + 1) * P], ident)
                                xi = xi2_pool.tile([P, C], F32, tag="xi")
                                nc.sync.dma_start(out=xi[:], in_=xpv[i])
                                xo = xo_pool.tile([P, C], F32, tag="xo")
                                nc.vector.tensor_tensor(out=xo[:], in0=xi[:], in1=tm[:], op=OP.add)
                                nc.sync.dma_start(out=outv[i], in_=xo[:])

        if repeat == 1:
            body(None)
        else:
            with tc.For_i(0, repeat, 1) as iv:
                body(iv)
    nc.compile()
    return nc


def prep_inputs(x_core, w_qkv, b_qkv, w_attn_proj, b_attn_proj, w_fc, b_fc,
                w_mlp_proj, b_mlp_proj, ln1_w, ln1_b, ln2_w, ln2_b):
    """Host-side layout prep for one core. x_core: [T, C] f32."""
    bf = ml_dtypes.bfloat16
    f32 = np.float32

    wqkvT = np.ascontiguousarray(w_qkv.T).astype(bf)                  # [C, 3C]
    # attn_proj weights grouped by head, zero-padded 96->128 rows
    wa = w_attn_proj.T.reshape(H, D, C)                                # [8, 96, 768]
    wa_pad = np.zeros((H, P, C), f32)
    wa_pad[:, 1:D + 1, :] = wa
    wa_pad = np.ascontiguousarray(wa_pad.reshape(H * P, C)).astype(bf)
    wfT = np.ascontiguousarray(w_fc.T).astype(bf)                      # [C, FC]
    wmT = np.ascontiguousarray(w_mlp_proj.T).astype(bf)                # [FC, C]

    bqk_cols = np.zeros((P, 16), f32)
    for m in range(16):
        src = b_qkv[m * D:(m + 1) * D] if m < 8 else b_qkv[C + (m - 8) * D:C + (m - 7) * D]
        bqk_cols[:D, m] = src
    bvb = np.broadcast_to(b_qkv[2 * C:3 * C][None, :], (P, C)).astype(f32)

    def cols(v, n):
        return np.ascontiguousarray(v.reshape(n, P).T).astype(f32)     # [P, n]

    tri = np.triu(np.ones((P, P), f32)).astype(bf)                     # keep f >= p
    ident = np.eye(P, dtype=f32).astype(bf)

    return {
        "x": np.ascontiguousarray(x_core, f32),
        "wqkvT": wqkvT, "waT": wa_pad, "wfT": wfT, "wmT": wmT,
        "bqk": bqk_cols, "bvb": np.ascontiguousarray(bvb),
        "ba": cols(b_attn_proj, NF), "bf": cols(b_fc, NFC), "bm": cols(b_mlp_proj, NF),
        "l1w": cols(ln1_w, NF), "l1b": cols(ln1_b, NF),
        "l2w": cols(ln2_w, NF), "l2b": cols(ln2_b, NF),
        "tri": tri, "ident": ident,
    }


_NC_CACHE = {}


def get_nc(repeat=1, debug=False):
    key = (repeat, debug)
    if key not in _NC_CACHE:
        _NC_CACHE[key] = build_nc(repeat=repeat, debug=debug)
    return _NC_CACHE[key]


def kernel(**inputs):
    from concourse.bass_utils import run_bass_kernel_spmd
    nc = get_nc()
    x = np.asarray(inputs["x"])
    B = x.shape[0]
    shared = None
    in_maps = []
    for b in range(B):
        m = prep_inputs(
            x[b], np.asarray(inputs["w_qkv"]), np.asarray(inputs["b_qkv"]),
            np.asarray(inputs["w_attn_proj"]), np.asarray(inputs["b_attn_proj"]),
            np.asarray(inputs["w_fc"]), np.asarray(inputs["b_fc"]),
            np.asarray(inputs["w_mlp_proj"]), np.asarray(inputs["b_mlp_proj"]),
            np.asarray(inputs["ln1_w"]), np.asarray(inputs["ln1_b"]),
            np.asarray(inputs["ln2_w"]), np.asarray(inputs["ln2_b"])) if shared is None else None
        if m is not None:
            shared = {k: v for k, v in m.items() if k != "x"}
            in_maps.append(m)
        else:
            mm = dict(shared)
            mm["x"] = np.ascontiguousarray(x[b], np.float32)
            in_maps.append(mm)
    r = run_bass_kernel_spmd(nc, in_maps, core_ids=list(range(B)))
    return np.stack([r.results[b]["out"] for b in range(B)], axis=0)


# revision 19
# speedup vs baseline: 1.0170x; 1.0170x over previous
"""Trainium2 Bass kernel for one dense transformer block.

B=8, T=2048, C=768, 8 heads. Data-parallel: one batch element per NeuronCore,
no collectives. Full inputs in, full output out.
"""
import os
import numpy as np
import ml_dtypes

import concourse.bass as bass
import concourse.mybir as mybir
import concourse.tile as tile
from concourse import bacc
from contextlib import ExitStack

F32 = mybir.dt.float32
BF16 = mybir.dt.bfloat16
AF = mybir.ActivationFunctionType
OP = mybir.AluOpType

P = 128
T, C = 2048, 768
H, D = 8, 96
FC = 3072
NT = T // P          # 16 token tiles
NF = C // P          # 6 feature chunks
NFC = FC // P        # 24 fc feature tiles
LN_EPS = 1e-5
SM_SCALE = 1.0 / float(np.sqrt(D))


def _ln_tile(nc, pools, xi):
    """LayerNorm stats + normalize for one [128, C] f32 tile -> bf16 tile.

    Returns the normalized (x - mean) * rstd tile (gamma/beta applied later,
    per-feature, during the post-transpose eviction).
    """
    st6 = pools["st6"].tile([P, 2, 6], F32, tag="st6")
    nc.vector.bn_stats(st6[:, 0, :], xi[:, 0:C // 2])
    nc.vector.bn_stats(st6[:, 1, :], xi[:, C // 2:C])
    aggr = pools["aggr"].tile([P, 2], F32, tag="aggr")
    nc.vector.bn_aggr(aggr[:], st6[:])
    std = pools["std"].tile([P, 1], F32, tag="std")
    nc.scalar.activation(std[:], aggr[:, 1:2], AF.Sqrt, bias=pools["eps"][:])
    rstd = pools["rstd"].tile([P, 1], F32, tag="rstd")
    nc.vector.reciprocal(rstd[:], std[:])
    xln = pools["xln"].tile([P, C], BF16, tag="xln")
    nc.vector.tensor_scalar(out=xln[:], in0=xi, scalar1=aggr[:, 0:1],
                            scalar2=rstd[:], op0=OP.subtract, op1=OP.mult)
    return xln


def _ln_transpose_group(nc, tc, pools, xlns, g, xt_dst, ident, lnw, lnb):
    """Transpose 4 normalized token tiles into xt_dst[:, f, g*512:(g+1)*512],
    applying per-feature gamma/beta during PSUM eviction."""
    for f in range(NF):
        pt = pools["tps"].tile([P, 512], BF16, tag="tps")
        for j in range(4):
            nc.tensor.transpose(pt[:, j * P:(j + 1) * P],
                                xlns[j][:, f * P:(f + 1) * P], ident)
        nc.vector.tensor_scalar(out=xt_dst[:, f, g * 512:(g + 1) * 512],
                                in0=pt[:], scalar1=lnw[:, f:f + 1],
                                scalar2=lnb[:, f:f + 1], op0=OP.mult, op1=OP.add)


def build_nc(repeat=1, debug=False):
    nc = bacc.Bacc(None, target_bir_lowering=False)
    with tile.TileContext(nc) as tc, ExitStack() as top:
        dram = top.enter_context(tc.tile_pool(name="dram", bufs=1, space="DRAM"))

        def din(name, shape, dt):
            return dram.tile(shape, dt, kind="ExternalInput", name=name, uniquify=False)

        x_d = din("x", [T, C], F32)
        wqkvT_d = din("wqkvT", [C, 3 * C], BF16)       # w_qkv.T
        wa_d = din("waT", [H * P, C], BF16)            # w_attn_proj.T head-padded
        wf_d = din("wfT", [C, FC], BF16)               # w_fc.T
        wm_d = din("wmT", [FC, C], BF16)               # w_mlp_proj.T
        bqk_d = din("bqk", [P, 16], F32)               # per head-block bias cols
        bvb_d = din("bvb", [P, C], F32)                # b_v broadcast
        ba_d = din("ba", [P, NF], F32)
        bf_d = din("bf", [P, NFC], F32)
        bm_d = din("bm", [P, NF], F32)
        l1w_d = din("l1w", [P, NF], F32)
        l1b_d = din("l1b", [P, NF], F32)
        l2w_d = din("l2w", [P, NF], F32)
        l2b_d = din("l2b", [P, NF], F32)
        tri_d = din("tri", [P, P], BF16)
        id_d = din("ident", [P, P], BF16)
        out_d = dram.tile([T, C], F32, kind="ExternalOutput", name="out", uniquify=False)
        xp_d = dram.tile([T, C], F32, name="xprime", uniquify=False)
        if debug:
            dbg_x1t = dram.tile([P, NF, T], BF16, kind="ExternalOutput", name="dbg_x1t", uniquify=False)
            dbg_qk = dram.tile([P, 16, T], BF16, kind="ExternalOutput", name="dbg_qk", uniquify=False)
            dbg_v = dram.tile([P, NT, H, 97], BF16, kind="ExternalOutput", name="dbg_v", uniquify=False)
            dbg_o = dram.tile([P, H, T], BF16, kind="ExternalOutput", name="dbg_o", uniquify=False)

        xv = x_d[:].rearrange("(n p) c -> n p c", p=P)
        xpv = xp_d[:].rearrange("(n p) c -> n p c", p=P)
        outv = out_d[:].rearrange("(n p) c -> n p c", p=P)

        # constants / params, loaded once
        cst = top.enter_context(tc.tile_pool(name="cst", bufs=1))
        ident = cst.tile([P, P], BF16)
        nc.sync.dma_start(out=ident[:], in_=id_d[:])
        tri = cst.tile([P, P], BF16)
        nc.sync.dma_start(out=tri[:], in_=tri_d[:])
        bqk = cst.tile([P, 16], F32)
        nc.sync.dma_start(out=bqk[:], in_=bqk_d[:])
        bvb = cst.tile([P, C], F32)
        nc.sync.dma_start(out=bvb[:], in_=bvb_d[:])
        ba = cst.tile([P, NF], F32)
        nc.sync.dma_start(out=ba[:], in_=ba_d[:])
        bf = cst.tile([P, NFC], F32)
        nc.sync.dma_start(out=bf[:], in_=bf_d[:])
        bm = cst.tile([P, NF], F32)
        nc.sync.dma_start(out=bm[:], in_=bm_d[:])
        l1w = cst.tile([P, NF], F32)
        nc.sync.dma_start(out=l1w[:], in_=l1w_d[:])
        l1b = cst.tile([P, NF], F32)
        nc.sync.dma_start(out=l1b[:], in_=l1b_d[:])
        l2w = cst.tile([P, NF], F32)
        nc.sync.dma_start(out=l2w[:], in_=l2w_d[:])
        l2b = cst.tile([P, NF], F32)
        nc.sync.dma_start(out=l2b[:], in_=l2b_d[:])
        eps_t = cst.tile([P, 1], F32)
        nc.vector.memset(eps_t[:], LN_EPS)

        def body(iv):
            def make_ln_pools(stack, sfx):
                return {
                    "st6": stack.enter_context(tc.tile_pool(name="st6" + sfx, bufs=4)),
                    "aggr": stack.enter_context(tc.tile_pool(name="aggr" + sfx, bufs=4)),
                    "std": stack.enter_context(tc.tile_pool(name="std" + sfx, bufs=4)),
                    "rstd": stack.enter_context(tc.tile_pool(name="rstd" + sfx, bufs=4)),
                    "xln": stack.enter_context(tc.tile_pool(name="xln" + sfx, bufs=6)),
                    "eps": eps_t,
                }

            with ExitStack() as es:
                xt_pool = es.enter_context(tc.tile_pool(name="xt", bufs=1))

                # ---------------- Phase A: LN1 + transpose ----------------
                x1t = xt_pool.tile([P, NF, T], BF16, tag="xt")
                with ExitStack() as ea:
                    ln_pools = make_ln_pools(ea, "a")
                    xi_pool = ea.enter_context(tc.tile_pool(name="xi_a", bufs=6))
                    ln_pools["tps"] = ea.enter_context(
                        tc.tile_pool(name="tps_a", bufs=2, space="PSUM"))
                    for g in range(4):
                        xlns = []
                        for j in range(4):
                            i = g * 4 + j
                            xi = xi_pool.tile([P, C], F32, tag="xi")
                            nc.sync.dma_start(out=xi[:], in_=xv[i])
                            xlns.append(_ln_tile(nc, ln_pools, xi[:]))
                        _ln_transpose_group(nc, tc, ln_pools, xlns, g, x1t, ident, l1w, l1b)

                # ---------------- Phase B: QKV ----------------
                es_bc = ExitStack()
                qk_pool = es_bc.enter_context(tc.tile_pool(name="qk", bufs=1))
                v_pool = es_bc.enter_context(tc.tile_pool(name="v", bufs=1))
                qk_sb = qk_pool.tile([P, 16, T], BF16, tag="qk")
                v_sb = v_pool.tile([P, NT, H, 97], BF16, tag="v")
                if debug:
                    nc.gpsimd.memset(qk_sb[96:128, :, :], 0.0)
                nc.vector.memset(v_sb[:, :, :, 0], 1.0)
                with ExitStack() as eb:
                    wq_pool = eb.enter_context(tc.tile_pool(name="wq", bufs=1))
                    qkps_pool = eb.enter_context(tc.tile_pool(name="qkps", bufs=4, space="PSUM"))
                    vps_pool = eb.enter_context(tc.tile_pool(name="vps", bufs=2, space="PSUM"))
                    wq = wq_pool.tile([P, NF, 3 * C], BF16, tag="wq")
                    nc.sync.dma_start(out=wq[:], in_=wqkvT_d[:].rearrange("(f p) c -> p f c", p=P))
                    # B1: Q, K feature-major, head-padded blocks (M=96)
                    for m in range(16):
                        col0 = m * D if m < 8 else C + (m - 8) * D
                        for n in range(4):
                            ps = qkps_pool.tile([P, 512], F32, tag="qkps")
                            for f in range(NF):
                                nc.tensor.matmul(ps[0:D, :], lhsT=wq[:, f, col0:col0 + D],
                                                 rhs=x1t[:, f, n * 512:(n + 1) * 512],
                                                 start=(f == 0), stop=(f == NF - 1))
                            nc.scalar.activation(qk_sb[0:D, m, n * 512:(n + 1) * 512],
                                                 ps[0:D, :], AF.Identity,
                                                 bias=bqk[0:D, m:m + 1])
                    # B2: V token-major [tk, h, d] + ones column
                    for i in range(NT):
                        ps = vps_pool.tile([P, C], F32, tag="vps")
                        for f in range(NF):
                            nc.tensor.matmul(ps[:, 0:512], lhsT=x1t[:, f, i * P:(i + 1) * P],
                                             rhs=wq[:, f, 2 * C:2 * C + 512],
                                             start=(f == 0), stop=(f == NF - 1))
                            nc.tensor.matmul(ps[:, 512:768], lhsT=x1t[:, f, i * P:(i + 1) * P],
                                             rhs=wq[:, f, 2 * C + 512:3 * C],
                                             start=(f == 0), stop=(f == NF - 1))
                        nc.vector.tensor_tensor(
                            out=v_sb[:, i, :, 1:97],
                            in0=ps[:].rearrange("p (h d) -> p h d", h=H),
                            in1=bvb[:].rearrange("p (h d) -> p h d", h=H),
                            op=OP.add)

                if debug:
                    nc.sync.dma_start(out=dbg_x1t[:], in_=x1t[:])
                    nc.sync.dma_start(out=dbg_qk[:], in_=qk_sb[:])
                    nc.sync.dma_start(out=dbg_v[:], in_=v_sb[:])

                # ---------------- Phase C: attention ----------------
                es_cd = ExitStack()
                o_pool = es_cd.enter_context(tc.tile_pool(name="o", bufs=1, side="right"))
                o_sb = o_pool.tile([P, H, T], BF16, tag="o")
                nc.scalar.memzero(o_sb[96:128, :, :])
                with ExitStack() as ec:
                    sps_pool = ec.enter_context(tc.tile_pool(name="sps", bufs=2, space="PSUM"))
                    ops_pool = ec.enter_context(tc.tile_pool(name="ops", bufs=1, space="PSUM"))
                    pt_pool = ec.enter_context(tc.tile_pool(name="pt", bufs=3))
                    r_pool = ec.enter_context(tc.tile_pool(name="r", bufs=1))
                    rb_pool = ec.enter_context(tc.tile_pool(name="rb", bufs=1))

                    stgo_pool = ec.enter_context(tc.tile_pool(name="stgo", bufs=2))

                    def emit_S(h, tk, pT):
                        q0 = tk * P
                        for l in range(q0 // 1024, 2):
                            sb_ = sps_pool.tile([P, 1024], F32, tag="sps")
                            for c2 in range(2):
                                cs = l * 1024 + c2 * 512
                                if cs + 512 <= q0:
                                    continue
                                nc.tensor.matmul(sb_[:, c2 * 512:(c2 + 1) * 512],
                                                 lhsT=qk_sb[0:D, 8 + h, q0:q0 + P],
                                                 rhs=qk_sb[0:D, h, cs:cs + 512],
                                                 start=True, stop=True)
                            e0 = max(q0 - l * 1024, 0)
                            nc.scalar.activation(pT[:, l * 1024 + e0:(l + 1) * 1024],
                                                 sb_[:, e0:1024], AF.Exp,
                                                 scale=SM_SCALE)
                        nc.vector.tensor_tensor(out=pT[:, q0:q0 + P], in0=pT[:, q0:q0 + P],
                                                in1=tri[:], op=OP.mult)

                    def emit_PV(h, tk, pT, o_ps):
                        q0 = tk * P
                        pos = q0
                        while pos < T:
                            bank = pos // 512
                            end = min((bank + 1) * 512, T)
                            nc.tensor.matmul(o_ps[0:97, pos:end],
                                             lhsT=v_sb[:, tk, h, :],
                                             rhs=pT[:, pos:end],
                                             start=(tk == 0),
                                             stop=(tk == min(4 * bank + 3, NT - 1)))
                            pos = end

                    for h in range(H):
                        o_ps = ops_pool.tile([P, T], F32, tag="ops")
                        pts = {}
                        for tk in range(NT):
                            pts[tk] = pt_pool.tile([P, T], BF16, tag="pt", name="pt")
                            emit_S(h, tk, pts[tk])
                            if tk >= 1:
                                emit_PV(h, tk - 1, pts[tk - 1], o_ps)
                                del pts[tk - 1]
                        emit_PV(h, NT - 1, pts[NT - 1], o_ps)
                        # free o_ps quickly: recip denom row + raw copy out
                        r_row = r_pool.tile([P, T], F32, tag="r")
                        nc.vector.reciprocal(r_row[0:1, :], o_ps[0:1, :])
                        stgo = stgo_pool.tile([P, T], BF16, tag="stgo")
                        nc.vector.tensor_copy(stgo[0:97, :], o_ps[0:97, :])
                        # normalize off the critical path
                        rb = rb_pool.tile([P, T], F32, tag="rb")
                        nc.gpsimd.partition_broadcast(rb[0:97, :], r_row[0:1, :], channels=97)
                        nc.vector.tensor_tensor(out=o_sb[0:97, h, :], in0=stgo[0:97, :],
                                                in1=rb[0:97, :], op=OP.mult)

                if debug:
                    nc.sync.dma_start(out=dbg_o[:], in_=o_sb[:])
                es_bc.close()

                # ---------------- Phase D: attn proj + residual + LN2 ----------------
                x2t = xt_pool.tile([P, NF, T], BF16, tag="xt")
                wf_pool = es.enter_context(tc.tile_pool(name="wf", bufs=1))
                wf = wf_pool.tile([P, NF, FC], BF16, tag="wf")
                with ExitStack() as ed:
                    wa_pool2 = ed.enter_context(tc.tile_pool(name="wa2", bufs=1))
                    wa2 = wa_pool2.tile([P, H, C], BF16, tag="wa2")
                    nc.sync.dma_start(out=wa2[:], in_=wa_d[:].rearrange("(h p) c -> p h c", p=P))
                    nc.sync.dma_start(out=wf[:], in_=wf_d[:].rearrange("(f p) c -> p f c", p=P))
                    prps_pool = ed.enter_context(tc.tile_pool(name="prps", bufs=2, space="PSUM"))
                    tmps_pool = ed.enter_context(tc.tile_pool(name="tmps", bufs=2, space="PSUM"))
                    stg_pool = ed.enter_context(tc.tile_pool(name="stg", bufs=2))
                    xi_pool = ed.enter_context(tc.tile_pool(name="xi_d", bufs=3))
                    xn_pool = ed.enter_context(tc.tile_pool(name="xn", bufs=6))
                    ln_pools2 = make_ln_pools(ed, "d")
                    ln_pools2["tps"] = ed.enter_context(
                        tc.tile_pool(name="tps2", bufs=2, space="PSUM"))

                    for n in range(4):
                        stg = stg_pool.tile([P, NF, 512], BF16, tag="stg")
                        for fo in range(NF):
                            ps = prps_pool.tile([P, 512], F32, tag="prps")
                            for h in range(H):
                                nc.tensor.matmul(ps[:], lhsT=wa2[:, h, fo * P:(fo + 1) * P],
                                                 rhs=o_sb[:, h, n * 512:(n + 1) * 512],
                                                 start=(h == 0), stop=(h == H - 1))
                            nc.scalar.activation(stg[:, fo, :], ps[:], AF.Identity,
                                                 bias=ba[:, fo:fo + 1])
                        xlns = []
                        for j in range(4):
                            i = n * 4 + j
                            tm = tmps_pool.tile([P, C], BF16, tag="tmps")
                            for fo in range(NF):
                                nc.tensor.transpose(tm[:, fo * P:(fo + 1) * P],
                                                    stg[:, fo, j * P:(j + 1) * P], ident)
                            xi = xi_pool.tile([P, C], F32, tag="xi")
                            nc.sync.dma_start(out=xi[:], in_=xv[i])
                            xn = xn_pool.tile([P, C], F32, tag="xn")
                            nc.vector.tensor_tensor(out=xn[:], in0=xi[:], in1=tm[:], op=OP.add)
                            nc.sync.dma_start(out=xpv[i], in_=xn[:])
                            xlns.append(_ln_tile(nc, ln_pools2, xn[:]))
                        _ln_transpose_group(nc, tc, ln_pools2, xlns, n, x2t, ident, l2w, l2b)
                es_cd.close()

                # ---------------- Phase E: MLP (T halves) ----------------
                with ExitStack() as ee:
                    wm_pool = ee.enter_context(tc.tile_pool(name="wm", bufs=1))
                    wm = wm_pool.tile([P, NFC, C], BF16, tag="wm")
                    nc.sync.dma_start(out=wm[:], in_=wm_d[:].rearrange("(f p) c -> p f c", p=P))
                    h_pool = ee.enter_context(tc.tile_pool(name="h", bufs=1))
                    fcps_pool = ee.enter_context(tc.tile_pool(name="fcps", bufs=2, space="PSUM"))
                    prps2_pool = ee.enter_context(tc.tile_pool(name="prps2", bufs=2, space="PSUM"))
                    tmps2_pool = ee.enter_context(tc.tile_pool(name="tmps2", bufs=1, space="PSUM"))
                    stg2_pool = ee.enter_context(tc.tile_pool(name="stg2", bufs=2))
                    xi2_pool = ee.enter_context(tc.tile_pool(name="xi_e", bufs=3))
                    xo_pool = ee.enter_context(tc.tile_pool(name="xo", bufs=3))

                    for t2 in range(2):
                        tbase = t2 * 1024
                        h_sb = h_pool.tile([P, NFC, 1024], BF16, tag="h")
                        for m in range(NFC):
                            ps = fcps_pool.tile([P, 1024], F32, tag="fcps")
                            for c2 in range(2):
                                for f in range(NF):
                                    nc.tensor.matmul(
                                        ps[:, c2 * 512:(c2 + 1) * 512],
                                        lhsT=wf[:, f, m * P:(m + 1) * P],
                                        rhs=x2t[:, f, tbase + c2 * 512:tbase + (c2 + 1) * 512],
                                        start=(f == 0), stop=(f == NF - 1))
                            nc.scalar.activation(h_sb[:, m, :], ps[:], AF.Gelu_apprx_tanh,
                                                 bias=bf[:, m:m + 1])
                        for n2 in range(2):
                            stg = stg2_pool.tile([P, NF, 512], BF16, tag="stg2")
                            for fo in range(NF):
                                ps = prps2_pool.tile([P, 512], F32, tag="prps2")
                                for k in range(NFC):
                                    nc.tensor.matmul(ps[:], lhsT=wm[:, k, fo * P:(fo + 1) * P],
                                                     rhs=h_sb[:, k, n2 * 512:(n2 + 1) * 512],
                                                     start=(k == 0), stop=(k == NFC - 1))
                                nc.scalar.activation(stg[:, fo, :], ps[:], AF.Identity,
                                                     bias=bm[:, fo:fo + 1])
                            for j in range(4):
                                i = t2 * 8 + n2 * 4 + j
                                tm = tmps2_pool.tile([P, C], BF16, tag="tmps2")
                                for fo in range(NF):
                                    nc.tensor.transpose(tm[:, fo * P:(fo + 1) * P],
                                                        stg[:, fo, j * P:(j + 1) * P], ident)
                                xi = xi2_pool.tile([P, C], F32, tag="xi")
                                nc.sync.dma_start(out=xi[:], in_=xpv[i])
                                xo = xo_pool.tile([P, C], F32, tag="xo")
                                nc.vector.tensor_tensor(out=xo[:], in0=xi[:], in1=tm[:], op=OP.add)
                                nc.sync.dma_start(out=outv[i], in_=xo[:])

        if repeat == 1:
            body(None)
        else:
            with tc.For_i(0, repeat, 1) as iv:
                body(iv)
    nc.compile()
    return nc


def prep_inputs(x_core, w_qkv, b_qkv, w_attn_proj, b_attn_proj, w_fc, b_fc,
                w_mlp_proj, b_mlp_proj, ln1_w, ln1_b, ln2_w, ln2_b):
    """Host-side layout prep for one core. x_core: [T, C] f32."""
    bf = ml_dtypes.bfloat16
    f32 = np.float32

    wqkvT = np.ascontiguousarray(w_qkv.T).astype(bf)                  # [C, 3C]
    # attn_proj weights grouped by head, zero-padded 96->128 rows
    wa = w_attn_proj.T.reshape(H, D, C)                                # [8, 96, 768]
    wa_pad = np.zeros((H, P, C), f32)
    wa_pad[:, 1:D + 1, :] = wa
    wa_pad = np.ascontiguousarray(wa_pad.reshape(H * P, C)).astype(bf)
    wfT = np.ascontiguousarray(w_fc.T).astype(bf)                      # [C, FC]
    wmT = np.ascontiguousarray(w_mlp_proj.T).astype(bf)                # [FC, C]

    bqk_cols = np.zeros((P, 16), f32)
    for m in range(16):
        src = b_qkv[m * D:(m + 1) * D] if m < 8 else b_qkv[C + (m - 8) * D:C + (m - 7) * D]
        bqk_cols[:D, m] = src
    bvb = np.broadcast_to(b_qkv[2 * C:3 * C][None, :], (P, C)).astype(f32)

    def cols(v, n):
        return np.ascontiguousarray(v.reshape(n, P).T).astype(f32)     # [P, n]

    tri = np.triu(np.ones((P, P), f32)).astype(bf)                     # keep f >= p
    ident = np.eye(P, dtype=f32).astype(bf)

    return {
        "x": np.ascontiguousarray(x_core, f32),
        "wqkvT": wqkvT, "waT": wa_pad, "wfT": wfT, "wmT": wmT,
        "bqk": bqk_cols, "bvb": np.ascontiguousarray(bvb),
        "ba": cols(b_attn_proj, NF), "bf": cols(b_fc, NFC), "bm": cols(b_mlp_proj, NF),
        "l1w": cols(ln1_w, NF), "l1b": cols(ln1_b, NF),
        "l2w": cols(ln2_w, NF), "l2b": cols(ln2_b, NF),
        "tri": tri, "ident": ident,
    }


_NC_CACHE = {}


def get_nc(repeat=1, debug=False):
    key = (repeat, debug)
    if key not in _NC_CACHE:
        _NC_CACHE[key] = build_nc(repeat=repeat, debug=debug)
    return _NC_CACHE[key]


def kernel(**inputs):
    from concourse.bass_utils import run_bass_kernel_spmd
    nc = get_nc()
    x = np.asarray(inputs["x"])
    B = x.shape[0]
    shared = None
    in_maps = []
    for b in range(B):
        m = prep_inputs(
            x[b], np.asarray(inputs["w_qkv"]), np.asarray(inputs["b_qkv"]),
            np.asarray(inputs["w_attn_proj"]), np.asarray(inputs["b_attn_proj"]),
            np.asarray(inputs["w_fc"]), np.asarray(inputs["b_fc"]),
            np.asarray(inputs["w_mlp_proj"]), np.asarray(inputs["b_mlp_proj"]),
            np.asarray(inputs["ln1_w"]), np.asarray(inputs["ln1_b"]),
            np.asarray(inputs["ln2_w"]), np.asarray(inputs["ln2_b"])) if shared is None else None
        if m is not None:
            shared = {k: v for k, v in m.items() if k != "x"}
            in_maps.append(m)
        else:
            mm = dict(shared)
            mm["x"] = np.ascontiguousarray(x[b], np.float32)
            in_maps.append(mm)
    r = run_bass_kernel_spmd(nc, in_maps, core_ids=list(range(B)))
    return np.stack([r.results[b]["out"] for b in range(B)], axis=0)
